# revision 2
# baseline (speedup 1.0000x reference)
"""Trainium2 Bass kernel for nn_LogicLayer (differentiable logic-gate layer).

Reference computation:
    a = x[:, idx_a]; b = x[:, idx_b]                  # [B, OUT] gathers
    w = softmax(weights, -1)                          # [OUT, 16]
    out = sum_k w[:, k] * gate_k(a, b)

Every gate value is of the form c0 + c1*a + c2*b + c3*a*b, so
    out[i, j] = W0[j] + W1[j]*a + W2[j]*b + W3[j]*a*b
with W = softmax(weights) @ C, C the [16, 4] gate-coefficient table.

Kernel strategy (data-parallel over batch across 8 cores, 256 rows/core):
  1. softmax+C projection on device -> W0..W3 tiles kept in SBUF
     (heavy reduces on GPSIMD so they overlap the x loads on DMA)
  2. PE-transpose the core's x shard [256, 8192] -> xT [8192, 256] in DRAM
     (stores batched 4 row-blocks per DMA to keep HWDGE off the critical path)
  3. dma_gather rows of xT for idx_a / idx_b (out_dim lands on partitions)
  4. u = W3*a + W2 (ACT), v = W1*a + W0 (DVE ts), t = u*b (DVE/Pool tt)
  5. out = t + v realized directly in PSUM by PE transpose-accumulate
     (two is_transpose matmuls into the same bank), copied back and stored
     in natural [256, 8192] layout.
"""

import numpy as np

# ---------------------------------------------------------------- constants
B_TOT, IN_DIM, OUT_DIM = 2048, 8192, 8192
NCORES = 8

# value = c0 + c1*a + c2*b + c3*ab  for each of the 16 gates
GATE_C = np.array(
    [
        # c0  c1  c2  c3
        [0, 0, 0, 0],    # 0  False
        [0, 0, 0, 1],    # 1  a AND b
        [0, 1, 0, -1],   # 2  a AND NOT b
        [0, 1, 0, 0],    # 3  a
        [0, 0, 1, -1],   # 4  NOT a AND b
        [0, 0, 1, 0],    # 5  b
        [0, 1, 1, -2],   # 6  a XOR b
        [0, 1, 1, -1],   # 7  a OR b
        [1, -1, -1, 1],  # 8  NOT (a OR b)
        [1, -1, -1, 2],  # 9  NOT (a XOR b)
        [1, 0, -1, 0],   # 10 NOT b
        [1, 0, -1, 1],   # 11 a OR NOT b
        [1, -1, 0, 0],   # 12 NOT a
        [1, -1, 0, 1],   # 13 NOT a OR b
        [1, 0, 0, -1],   # 14 NOT (a AND b)
        [1, 0, 0, 0],    # 15 True
    ],
    dtype=np.float32,
)  # [16, 4]


# ---------------------------------------------------------------- device IR
def build_nc(B=B_TOT // NCORES, IN=IN_DIM, OUT=OUT_DIM, NJ=1024):
    """Build the per-core Bass module (SPMD; all cores run the same IR)."""
    import sys

    if "/opt/trn_rl_repo" not in sys.path:
        sys.path.insert(0, "/opt/trn_rl_repo")

    import concourse.tile as tile
    from concourse import bacc, mybir
    from concourse.masks import make_identity
    from contextlib import ExitStack

    f32 = mybir.dt.float32
    i16 = mybir.dt.int16
    PB = B // 128          # batch partition-blocks
    NCH = OUT // NJ        # out_dim chunks
    SLOTS = NJ // 128      # 128-wide j-slots per chunk
    RPT = OUT // 128       # = NCH * SLOTS  (W free dim per partition)
    LCH = min(IN, 2048)    # x load chunk (columns)
    SG = 4                 # xT row-blocks batched per store

    nc = bacc.Bacc("TRN2", target_bir_lowering=False)
    x = nc.declare_dram_parameter("x", [B, IN], f32, isOutput=False)
    wgt = nc.declare_dram_parameter("wgt_shuf", [128, RPT * 16], f32, isOutput=False)
    cg = nc.declare_dram_parameter("cgate", [128, 64], f32, isOutput=False)
    idxa = nc.declare_dram_parameter("idxa16", [128, OUT // 16], i16, isOutput=False)
    idxb = nc.declare_dram_parameter("idxb16", [128, OUT // 16], i16, isOutput=False)
    out = nc.declare_dram_parameter("out", [B, OUT], f32, isOutput=True)

    Ident = mybir.ActivationFunctionType.Identity
    Exp = mybir.ActivationFunctionType.Exp
    MULT = mybir.AluOpType.mult
    ADD = mybir.AluOpType.add

    with tile.TileContext(nc) as tc, ExitStack() as ctx:
        dram = ctx.enter_context(tc.tile_pool(name="dram", bufs=1, space="DRAM"))
        xT = dram.tile([IN, B], f32, name="xT")

        cpool = ctx.enter_context(tc.tile_pool(name="consts", bufs=1))
        xs_stack = ExitStack()
        xs_pool = xs_stack.enter_context(tc.tile_pool(name="xs", bufs=1, side="right"))
        idx_pool = ctx.enter_context(tc.tile_pool(name="idxp", bufs=1))

        # wgt first (small) so the W-phase chain starts immediately,
        # then x shard loads saturate DMA while W-phase compute runs
        cgt = cpool.tile([128, 64], f32, name="cgt")
        nc.sync.dma_start(cgt[:], cg[:])
        wpool = ctx.enter_context(tc.tile_pool(name="wtmp", bufs=2))
        wtile = wpool.tile([128, RPT * 16], f32, name="wtile")
        nc.sync.dma_start(wtile[:], wgt[:])
        xh = {}
        for c0 in range(IN // LCH):
            for h in range(PB):
                xht = xs_pool.tile([128, LCH], f32, name=f"xh{h}_{c0}",
                                   tag=f"xh{h}_{c0}")
                nc.sync.dma_start(xht[:], x[h * 128:(h + 1) * 128,
                                            c0 * LCH:(c0 + 1) * LCH])
                xh[h, c0] = xht
        idxa_sb = idx_pool.tile([128, OUT // 16], i16, name="idxa_sb")
        nc.sync.dma_start(idxa_sb[:], idxa[:])
        idxb_sb = idx_pool.tile([128, OUT // 16], i16, name="idxb_sb")
        nc.sync.dma_start(idxb_sb[:], idxb[:])

        ident = cpool.tile([128, 128], f32, name="ident")
        make_identity(nc, ident[:])

        # ---- W = softmax(weights) @ C, in (q, r) layout: j = r*128 + q ----
        # heavy elementwise on GPSIMD so DVE stays free for phase-A copybacks
        wk = [cpool.tile([128, RPT], f32, name=f"wk{k}") for k in range(4)]
        if True:
            wexp = wpool.tile([128, RPT * 16], f32, name="wexp")
            nc.scalar.activation(wexp[:], wtile[:], Exp)
            wsum = wpool.tile([128, RPT], f32, name="wsum")
            nc.vector.tensor_reduce(
                out=wsum[:],
                in_=wexp[:].rearrange("p (r k) -> p r k", k=16),
                op=ADD,
                axis=mybir.AxisListType.X,
            )
            wrcp = wpool.tile([128, RPT], f32, name="wrcp")
            nc.vector.reciprocal(wrcp[:], wsum[:])
            for k in range(4):
                wtmp = wpool.tile([128, RPT * 16], f32, name="wtmp", tag="wtmp")
                ck_bcast = (
                    cgt[:, k * 16:(k + 1) * 16]
                    .rearrange("p (r k) -> p r k", r=1)
                    .to_broadcast([128, RPT, 16])
                )
                nc.gpsimd.tensor_tensor(
                    out=wtmp[:].rearrange("p (r k) -> p r k", k=16),
                    in0=wexp[:].rearrange("p (r k) -> p r k", k=16),
                    in1=ck_bcast,
                    op=MULT,
                )
                wred = wpool.tile([128, RPT], f32, name="wred", tag="wred")
                nc.vector.tensor_reduce(
                    out=wred[:],
                    in_=wtmp[:].rearrange("p (r k) -> p r k", k=16),
                    op=ADD,
                    axis=mybir.AxisListType.X,
                )
                nc.vector.tensor_tensor(out=wk[k][:], in0=wred[:], in1=wrcp[:],
                                        op=MULT)

        # ---- phase A: transpose x shard into xT (DRAM) ----
        psumT = ctx.enter_context(tc.tile_pool(name="psumT", bufs=4, space="PSUM"))
        stg_pool = ctx.enter_context(tc.tile_pool(name="xstg", bufs=3))
        if True:
            for g in range(IN // (SG * 128)):
                st = stg_pool.tile([128, SG, B], f32, tag="st")
                for i in range(SG):
                    cb = g * SG + i
                    c0, cc = (cb * 128) // LCH, (cb * 128) % LCH
                    pt = psumT.tile([128, B], f32, tag="pt")
                    for h in range(PB):
                        nc.tensor.transpose(
                            pt[:, h * 128:(h + 1) * 128],
                            xh[h, c0][:, cc:cc + 128],
                            ident[:],
                        )
                    if cb % 2 == 0:
                        nc.vector.tensor_copy(st[:, i, :], pt[:])
                    else:
                        nc.scalar.copy(st[:, i, :], pt[:])
                nc.sync.dma_start(
                    xT[g * SG * 128:(g + 1) * SG * 128, :]
                    .rearrange("(i p) b -> p i b", p=128),
                    st[:],
                )

        xs_stack.close()  # release x tiles; phase-B pools reuse the zone

        # ---- phase B: gather + gates + transpose-back ----
        gpool = ctx.enter_context(tc.tile_pool(name="gath", bufs=4))
        uvpool = ctx.enter_context(tc.tile_pool(name="uv", bufs=12))
        psumO = ctx.enter_context(tc.tile_pool(name="psumO", bufs=4, space="PSUM"))
        ostg = ctx.enter_context(tc.tile_pool(name="ostg", bufs=3))
        if True:
            NJ16 = NJ // 16
            for ck in range(NCH):
                ga = gpool.tile([128, SLOTS, B], f32, tag="ga")
                nc.gpsimd.dma_gather(
                    ga[:], xT[:], idxa_sb[:, ck * NJ16:(ck + 1) * NJ16], NJ, NJ, B
                )
                gb = gpool.tile([128, SLOTS, B], f32, tag="gb")
                nc.gpsimd.dma_gather(
                    gb[:], xT[:], idxb_sb[:, ck * NJ16:(ck + 1) * NJ16], NJ, NJ, B
                )
                for cq in range(SLOTS // 4):
                    ts_v, ts_t = [], []
                    for ci in range(4):
                        c = cq * 4 + ci
                        r = ck * SLOTS + c
                        u = uvpool.tile([128, B], f32, tag="u")
                        nc.scalar.activation(
                            u[:], ga[:, c, :], Ident,
                            scale=wk[3][:, r:r + 1], bias=wk[2][:, r:r + 1],
                        )
                        v = uvpool.tile([128, B], f32, tag="v")
                        nc.vector.tensor_scalar(
                            v[:], ga[:, c, :],
                            wk[1][:, r:r + 1], wk[0][:, r:r + 1],
                            op0=MULT, op1=ADD,
                        )
                        t = uvpool.tile([128, B], f32, tag="t")
                        eng = nc.gpsimd if ci == 3 else nc.vector
                        eng.tensor_tensor(t[:], u[:], gb[:, c, :], op=MULT)
                        ts_v.append(v)
                        ts_t.append(t)
                    for h in range(PB):
                        po = psumO.tile([128, 512], f32, tag="po")
                        for ci in range(4):
                            hs = slice(h * 128, (h + 1) * 128)
                            nc.tensor.matmul(
                                po[:, ci * 128:(ci + 1) * 128],
                                ts_t[ci][:, hs], ident[:],
                                is_transpose=True, start=True, stop=False,
                            )
                            nc.tensor.matmul(
                                po[:, ci * 128:(ci + 1) * 128],
                                ts_v[ci][:, hs], ident[:],
                                is_transpose=True, start=False, stop=True,
                            )
                        og = ostg.tile([128, 512], f32, tag="og")
                        if (h + cq) % 2 == 0:
                            nc.vector.tensor_copy(og[:], po[:])
                        else:
                            nc.scalar.copy(og[:], po[:])
                        j0 = ck * NJ + cq * 512
                        nc.sync.dma_start(
                            out[h * 128:(h + 1) * 128, j0:j0 + 512], og[:]
                        )
    nc.compile()
    return nc


# ---------------------------------------------------------------- host side
def _wrap_idx(idx, OUT, NJ):
    """Pack an index vector into dma_gather's wrapped int16 layout.

    Per chunk ck the NJ indices live in columns [ck*NJ/16, (ck+1)*NJ/16):
    idx16[p, ck*NJ/16 + s] = idx[ck*NJ + s*16 + p%16], replicated over the
    8 groups of 16 partitions.
    """
    nch = OUT // NJ
    a = np.asarray(idx).astype(np.int16).reshape(nch, NJ // 16, 16)  # [ck, s, p]
    a = a.transpose(2, 0, 1).reshape(16, nch * (NJ // 16))           # [p, ck*s]
    return np.ascontiguousarray(np.tile(a, (8, 1)))                  # [128, ...]


def _prep_inputs(x, weights, idx_a, idx_b, NJ=1024):
    B = B_TOT // NCORES
    NCH = OUT_DIM // NJ
    SLOTS = NJ // 128
    x = np.asarray(x, dtype=np.float32)
    weights = np.asarray(weights, dtype=np.float32)
    # wgt_shuf[q, (ck*SLOTS+c)*16+k] = weights[ck*NJ + c*128 + q, k]
    wgt_shuf = np.ascontiguousarray(
        weights.reshape(NCH, SLOTS, 128, 16).transpose(2, 0, 1, 3).reshape(128, -1)
    )
    cgate = np.ascontiguousarray(np.tile(GATE_C.T.reshape(1, 64), (128, 1)))
    ia = _wrap_idx(idx_a, OUT_DIM, NJ)
    ib = _wrap_idx(idx_b, OUT_DIM, NJ)
    in_maps = []
    for c in range(NCORES):
        in_maps.append(
            {
                "x": np.ascontiguousarray(x[c * B:(c + 1) * B]),
                "wgt_shuf": wgt_shuf,
                "cgate": cgate,
                "idxa16": ia,
                "idxb16": ib,
            }
        )
    return in_maps


_NC_CACHE = {}


def _get_nc():
    if "nc" not in _NC_CACHE:
        _NC_CACHE["nc"] = build_nc()
    return _NC_CACHE["nc"]


def _post(res, inputs=None):
    return np.concatenate([r["out"] for r in res.results], axis=0)


def kernel(x, weights, idx_a, idx_b):
    import sys

    if "/opt/trn_rl_repo" not in sys.path:
        sys.path.insert(0, "/opt/trn_rl_repo")
    from concourse.bass_utils import run_bass_kernel_spmd

    nc = _get_nc()
    in_maps = _prep_inputs(x, weights, idx_a, idx_b)
    res = run_bass_kernel_spmd(nc, in_maps, list(range(NCORES)))
    return _post(res)


if __name__ == "__main__":
    nc = build_nc()
    print("built OK")



# revision 7
# speedup vs baseline: 7.3551x; 7.3551x over previous
"""Trainium2 Bass kernel for nn_LogicLayer (differentiable logic-gate layer).

Reference computation:
    a = x[:, idx_a]; b = x[:, idx_b]                  # [B, OUT] gathers
    w = softmax(weights, -1)                          # [OUT, 16]
    out = sum_k w[:, k] * gate_k(a, b)

Every gate value is of the form c0 + c1*a + c2*b + c3*a*b, so
    out[i, j] = W0[j] + W1[j]*a + W2[j]*b + W3[j]*a*b
with W = softmax(weights) @ C, C the [16, 4] gate-coefficient table.

Kernel strategy (out_dim-parallel across 8 cores, 1024 neurons/core):
  host: W coefficients (softmax @ C, tiny), x transposed+cast to fp16
        xT16 [IN, B] passed as the gather table, per-core idx packing.
  device (per core, its 1024 j's, full batch on the free axis):
    1. dma_gather rows xT16[idx_a[j], :] and xT16[idx_b[j], :]
       (j on partitions, 4 KiB per gathered row -> efficient SWDGE DMA)
    2. s = W3*b + W1 (ACT), q = W2*b + W0 (DVE ts, 4x fp16 mode),
       m = a*s (DVE tt), o = m + q (DVE tt)
    3. store o to outT [1024, B] fp16 (4 KiB partition lines)
  host: assemble outT -> transpose -> float32 full output.

No PE/PSUM use at all and ~12 MiB HBM traffic per core vs ~41 MiB for
the batch-parallel transpose-on-device variant.
"""

import numpy as np

# ---------------------------------------------------------------- constants
B_TOT, IN_DIM, OUT_DIM = 2048, 8192, 8192
NCORES = 8
NJ_CORE = OUT_DIM // NCORES     # 1024 output neurons per core
CHUNK = 256                     # idxs per dma_gather call (2 slots of 128)

# value = c0 + c1*a + c2*b + c3*ab  for each of the 16 gates
GATE_C = np.array(
    [
        # c0  c1  c2  c3
        [0, 0, 0, 0],    # 0  False
        [0, 0, 0, 1],    # 1  a AND b
        [0, 1, 0, -1],   # 2  a AND NOT b
        [0, 1, 0, 0],    # 3  a
        [0, 0, 1, -1],   # 4  NOT a AND b
        [0, 0, 1, 0],    # 5  b
        [0, 1, 1, -2],   # 6  a XOR b
        [0, 1, 1, -1],   # 7  a OR b
        [1, -1, -1, 1],  # 8  NOT (a OR b)
        [1, -1, -1, 2],  # 9  NOT (a XOR b)
        [1, 0, -1, 0],   # 10 NOT b
        [1, 0, -1, 1],   # 11 a OR NOT b
        [1, -1, 0, 0],   # 12 NOT a
        [1, -1, 0, 1],   # 13 NOT a OR b
        [1, 0, 0, -1],   # 14 NOT (a AND b)
        [1, 0, 0, 0],    # 15 True
    ],
    dtype=np.float64,
)  # [16, 4]


# ---------------------------------------------------------------- device IR
def build_nc(NJ=NJ_CORE, IN=IN_DIM, B=B_TOT):
    """Build the per-core Bass module (SPMD; all cores run the same IR)."""
    import sys

    if "/opt/trn_rl_repo" not in sys.path:
        sys.path.insert(0, "/opt/trn_rl_repo")

    import concourse.tile as tile
    from concourse import bacc, mybir
    from contextlib import ExitStack

    f32 = mybir.dt.float32
    f16 = mybir.dt.float16
    i16 = mybir.dt.int16
    SLOTS = NJ // 128          # 8 j-slots per core
    # tapered chunk plan: big chunks first (dense DMA), small last (short
    # un-overlapped tail); desc-gen on GPSIMD is ~per-idx so total is flat
    CHUNKS = [512, 256, 128, 128]
    assert sum(CHUNKS) == NJ

    nc = bacc.Bacc("TRN2", target_bir_lowering=False)
    xt = nc.declare_dram_parameter("xt16", [IN, B], f16, isOutput=False)
    wc = nc.declare_dram_parameter("wcoef", [128, 4 * SLOTS], f32, isOutput=False)
    ia = nc.declare_dram_parameter("idxa16", [128, NJ // 16], i16, isOutput=False)
    ib = nc.declare_dram_parameter("idxb16", [128, NJ // 16], i16, isOutput=False)
    outt = nc.declare_dram_parameter("outt", [NJ, B], f16, isOutput=True)

    Ident = mybir.ActivationFunctionType.Identity
    MULT = mybir.AluOpType.mult
    ADD = mybir.AluOpType.add

    with tile.TileContext(nc) as tc, ExitStack() as ctx:
        cpool = ctx.enter_context(tc.tile_pool(name="consts", bufs=1))
        iat = cpool.tile([128, NJ // 16], i16, name="iat")
        nc.sync.dma_start(iat[:], ia[:])
        ibt = cpool.tile([128, NJ // 16], i16, name="ibt")
        nc.sync.dma_start(ibt[:], ib[:])
        wct = cpool.tile([128, 4 * SLOTS], f32, name="wct")
        nc.sync.dma_start(wct[:], wc[:])

        # one MOVE per distinct chunk size instead of one per gather call
        # (each MOVE costs ~0.4us of GPSIMD sequencer time up front)
        nregs = {n: nc.gpsimd.to_reg(n) for n in sorted(set(CHUNKS))}

        gpool = ctx.enter_context(tc.tile_pool(name="gath", bufs=1))
        spool = ctx.enter_context(tc.tile_pool(name="sqm", bufs=3))
        opool = ctx.enter_context(tc.tile_pool(name="out", bufs=4))

        def wap(k, c):  # [128, 1] f32 per-partition scalar for W_k, slot c
            return wct[:, k * SLOTS + c:k * SLOTS + c + 1]

        off = 0
        for ci, n in enumerate(CHUNKS):
            sl_n = n // 128
            icol0, icol1 = off // 16, (off + n) // 16
            ga = gpool.tile([128, sl_n, B], f16, tag=f"ga{ci}")
            nc.gpsimd.dma_gather(
                ga[:], xt[:], iat[:, icol0:icol1], n, nregs[n], B
            )
            gb = gpool.tile([128, sl_n, B], f16, tag=f"gb{ci}")
            nc.gpsimd.dma_gather(
                gb[:], xt[:], ibt[:, icol0:icol1], n, nregs[n], B
            )
            for sl in range(sl_n):
                c = off // 128 + sl
                s = spool.tile([128, B], f16, tag="s")
                nc.scalar.activation(
                    s[:], gb[:, sl, :], Ident, scale=wap(3, c), bias=wap(1, c)
                )
                q = spool.tile([128, B], f16, tag="q")
                nc.vector.tensor_scalar(
                    q[:], gb[:, sl, :], wap(2, c), wap(0, c), op0=MULT, op1=ADD
                )
                m = spool.tile([128, B], f16, tag="m")
                nc.vector.tensor_tensor(m[:], ga[:, sl, :], s[:], op=MULT)
                o = opool.tile([128, B], f16, tag="o")
                nc.vector.tensor_tensor(o[:], m[:], q[:], op=ADD)
                nc.sync.dma_start(outt[c * 128:(c + 1) * 128, :], o[:])
            off += n
    nc.compile()
    return nc


# ---------------------------------------------------------------- host side
def _wrap_idx(idx):
    """Pack an index vector into dma_gather's wrapped int16 layout:
    idx16[p, s] = idx[s*16 + p%16], replicated over the 8 groups of 16
    partitions."""
    n = len(idx)
    a = np.asarray(idx).astype(np.int16).reshape(n // 16, 16)  # [s, p]
    return np.ascontiguousarray(np.tile(a.T, (8, 1)))          # [128, n//16]


def _prep_inputs(x, weights, idx_a, idx_b):
    x = np.asarray(x, dtype=np.float32)
    w = np.asarray(weights, dtype=np.float64)
    e = np.exp(w - w.max(axis=-1, keepdims=True))
    sm = e / e.sum(axis=-1, keepdims=True)
    W4 = (sm @ GATE_C).astype(np.float32)                      # [OUT, 4]

    xt16 = x.T.astype(np.float16, order="C")                   # [IN, B]
    idx_a = np.asarray(idx_a)
    idx_b = np.asarray(idx_b)

    SLOTS = NJ_CORE // 128
    in_maps = []
    for c in range(NCORES):
        j0 = c * NJ_CORE
        # wcoef[q, k*SLOTS + c] = W4[j0 + c*128 + q, k]
        wcoef = np.ascontiguousarray(
            W4[j0:j0 + NJ_CORE]
            .reshape(SLOTS, 128, 4)
            .transpose(1, 2, 0)
            .reshape(128, 4 * SLOTS)
        )
        in_maps.append(
            {
                "xt16": xt16,
                "wcoef": wcoef,
                "idxa16": _wrap_idx(idx_a[j0:j0 + NJ_CORE]),
                "idxb16": _wrap_idx(idx_b[j0:j0 + NJ_CORE]),
            }
        )
    return in_maps


_NC_CACHE = {}


def _get_nc():
    if "nc" not in _NC_CACHE:
        _NC_CACHE["nc"] = build_nc()
    return _NC_CACHE["nc"]


def _post(res, inputs=None):
    outt = np.concatenate([r["outt"] for r in res.results], axis=0)  # [OUT, B]
    return outt.T.astype(np.float32, order="C")


def kernel(x, weights, idx_a, idx_b):
    import sys

    if "/opt/trn_rl_repo" not in sys.path:
        sys.path.insert(0, "/opt/trn_rl_repo")
    from concourse.bass_utils import run_bass_kernel_spmd

    nc = _get_nc()
    in_maps = _prep_inputs(x, weights, idx_a, idx_b)
    res = run_bass_kernel_spmd(nc, in_maps, list(range(NCORES)))
    return _post(res)


if __name__ == "__main__":
    nc = build_nc()
    print("built OK")


# revision 11
# speedup vs baseline: 7.8252x; 1.0639x over previous
"""Trainium2 Bass kernel for nn_LogicLayer (differentiable logic-gate layer).

Reference computation:
    a = x[:, idx_a]; b = x[:, idx_b]                  # [B, OUT] gathers
    w = softmax(weights, -1)                          # [OUT, 16]
    out = sum_k w[:, k] * gate_k(a, b)

Every gate value is of the form c0 + c1*a + c2*b + c3*a*b, so
    out[i, j] = W0[j] + W1[j]*a + W2[j]*b + W3[j]*a*b
with W = softmax(weights) @ C, C the [16, 4] gate-coefficient table.

Kernel strategy (out_dim-parallel across 8 cores, 1024 neurons/core):
  host: W coefficients (softmax @ C, tiny), x transposed+cast to fp16
        xT16 [IN, B] passed as the gather table, per-core idx packing.
  device (per core, its 1024 j's, full batch on the free axis):
    1. dma_gather rows xT16[idx_a[j], :] and xT16[idx_b[j], :]
       (j on partitions, 4 KiB per gathered row -> efficient SWDGE DMA)
    2. s = W3*b + W1 (ACT), q = W2*b + W0 (DVE ts, 4x fp16 mode),
       m = a*s (DVE tt), o = m + q (DVE tt)
    3. store o to outT [1024, B] fp16 (4 KiB partition lines)
  host: assemble outT -> transpose -> float32 full output.

No PE/PSUM use at all and ~12 MiB HBM traffic per core vs ~41 MiB for
the batch-parallel transpose-on-device variant.
"""

import numpy as np

# ---------------------------------------------------------------- constants
B_TOT, IN_DIM, OUT_DIM = 2048, 8192, 8192
NCORES = 8
NJ_CORE = OUT_DIM // NCORES     # 1024 output neurons per core
CHUNK = 256                     # idxs per dma_gather call (2 slots of 128)

# value = c0 + c1*a + c2*b + c3*ab  for each of the 16 gates
GATE_C = np.array(
    [
        # c0  c1  c2  c3
        [0, 0, 0, 0],    # 0  False
        [0, 0, 0, 1],    # 1  a AND b
        [0, 1, 0, -1],   # 2  a AND NOT b
        [0, 1, 0, 0],    # 3  a
        [0, 0, 1, -1],   # 4  NOT a AND b
        [0, 0, 1, 0],    # 5  b
        [0, 1, 1, -2],   # 6  a XOR b
        [0, 1, 1, -1],   # 7  a OR b
        [1, -1, -1, 1],  # 8  NOT (a OR b)
        [1, -1, -1, 2],  # 9  NOT (a XOR b)
        [1, 0, -1, 0],   # 10 NOT b
        [1, 0, -1, 1],   # 11 a OR NOT b
        [1, -1, 0, 0],   # 12 NOT a
        [1, -1, 0, 1],   # 13 NOT a OR b
        [1, 0, 0, -1],   # 14 NOT (a AND b)
        [1, 0, 0, 0],    # 15 True
    ],
    dtype=np.float64,
)  # [16, 4]


# ---------------------------------------------------------------- device IR
def build_nc(NJ=NJ_CORE, IN=IN_DIM, B=B_TOT):
    """Build the per-core Bass module (SPMD; all cores run the same IR)."""
    import sys

    if "/opt/trn_rl_repo" not in sys.path:
        sys.path.insert(0, "/opt/trn_rl_repo")

    import concourse.tile as tile
    from concourse import bacc, mybir, library_config
    from contextlib import ExitStack

    f32 = mybir.dt.float32
    f16 = mybir.dt.float16
    i16 = mybir.dt.int16
    SLOTS = NJ // 128          # 8 j-slots per core
    # small first chunk -> compute ramps early; small last -> short tail
    CHUNKS = [128, 256, 256, 256, 128]
    assert sum(CHUNKS) == NJ

    nc = bacc.Bacc("TRN2", target_bir_lowering=False)
    xt = nc.declare_dram_parameter("xt16", [IN, B], f16, isOutput=False)
    wc = nc.declare_dram_parameter("wcoef", [128, 4 * SLOTS], f32, isOutput=False)
    ia = nc.declare_dram_parameter("idxa16", [128, NJ // 16], i16, isOutput=False)
    ib = nc.declare_dram_parameter("idxb16", [128, NJ // 16], i16, isOutput=False)
    outt = nc.declare_dram_parameter("outt", [NJ, B], f16, isOutput=True)

    Ident = mybir.ActivationFunctionType.Identity
    MULT = mybir.AluOpType.mult
    ADD = mybir.AluOpType.add

    with tile.TileContext(nc) as tc, ExitStack() as ctx:
        # kick the Q7 gather-lib swap off as early as possible: its ~9us
        # load latency gates the first dma_gather desc-gen
        nc.gpsimd.load_library(library_config.mlp)

        cpool = ctx.enter_context(tc.tile_pool(name="consts", bufs=1))
        iat = cpool.tile([128, NJ // 16], i16, name="iat")
        nc.sync.dma_start(iat[:], ia[:])
        ibt = cpool.tile([128, NJ // 16], i16, name="ibt")
        nc.sync.dma_start(ibt[:], ib[:])
        wct = cpool.tile([128, 4 * SLOTS], f32, name="wct")
        nc.sync.dma_start(wct[:], wc[:])

        # one MOVE per distinct chunk size instead of one per gather call
        # (each MOVE costs ~0.4us of GPSIMD sequencer time up front)
        nregs = {n: nc.gpsimd.to_reg(n) for n in sorted(set(CHUNKS))}

        gpool = ctx.enter_context(tc.tile_pool(name="gath", bufs=1))
        spool = ctx.enter_context(tc.tile_pool(name="sqm", bufs=3))
        opool = ctx.enter_context(tc.tile_pool(name="out", bufs=4))

        def wap(k, c):  # [128, 1] f32 per-partition scalar for W_k, slot c
            return wct[:, k * SLOTS + c:k * SLOTS + c + 1]

        ACT_Q_SLOTS = {5, 6}   # late slots: ACT has idle time there, DVE not
        last_c = NJ // 128 - 1

        off = 0
        for ci, n in enumerate(CHUNKS):
            sl_n = n // 128
            icol0, icol1 = off // 16, (off + n) // 16
            # b feeds both s and q -> gather it first
            gb = gpool.tile([128, sl_n, B], f16, tag=f"gb{ci}")
            nc.gpsimd.dma_gather(
                gb[:], xt[:], ibt[:, icol0:icol1], n, nregs[n], B
            )
            ga = gpool.tile([128, sl_n, B], f16, tag=f"ga{ci}")
            nc.gpsimd.dma_gather(
                ga[:], xt[:], iat[:, icol0:icol1], n, nregs[n], B
            )
            for sl in range(sl_n):
                c = off // 128 + sl
                # final slot: split by batch halves to shorten the
                # un-overlapped dependence chain after the last gather
                hsplit = [slice(0, B // 2), slice(B // 2, B)] if c == last_c \
                    else [slice(0, B)]
                for hi, hs in enumerate(hsplit):
                    hb = hs.stop - hs.start
                    s = spool.tile([128, B], f16, tag="s")
                    nc.scalar.activation(
                        s[:, :hb], gb[:, sl, hs], Ident,
                        scale=wap(3, c), bias=wap(1, c),
                    )
                    q = spool.tile([128, B], f16, tag="q")
                    if c in ACT_Q_SLOTS:
                        nc.scalar.activation(
                            q[:, :hb], gb[:, sl, hs], Ident,
                            scale=wap(2, c), bias=wap(0, c),
                        )
                    else:
                        nc.vector.tensor_scalar(
                            q[:, :hb], gb[:, sl, hs], wap(2, c), wap(0, c),
                            op0=MULT, op1=ADD,
                        )
                    m = spool.tile([128, B], f16, tag="m")
                    nc.vector.tensor_tensor(
                        m[:, :hb], ga[:, sl, hs], s[:, :hb], op=MULT
                    )
                    o = opool.tile([128, B], f16, tag=f"o{hi}")
                    nc.vector.tensor_tensor(
                        o[:, :hb], m[:, :hb], q[:, :hb], op=ADD
                    )
                    nc.sync.dma_start(
                        outt[c * 128:(c + 1) * 128, hs], o[:, :hb]
                    )
            off += n
    nc.compile()
    return nc


# ---------------------------------------------------------------- host side
def _wrap_idx(idx):
    """Pack an index vector into dma_gather's wrapped int16 layout:
    idx16[p, s] = idx[s*16 + p%16], replicated over the 8 groups of 16
    partitions."""
    n = len(idx)
    a = np.asarray(idx).astype(np.int16).reshape(n // 16, 16)  # [s, p]
    return np.ascontiguousarray(np.tile(a.T, (8, 1)))          # [128, n//16]


def _prep_inputs(x, weights, idx_a, idx_b):
    x = np.asarray(x, dtype=np.float32)
    w = np.asarray(weights, dtype=np.float64)
    e = np.exp(w - w.max(axis=-1, keepdims=True))
    sm = e / e.sum(axis=-1, keepdims=True)
    W4 = (sm @ GATE_C).astype(np.float32)                      # [OUT, 4]

    xt16 = x.T.astype(np.float16, order="C")                   # [IN, B]
    idx_a = np.asarray(idx_a)
    idx_b = np.asarray(idx_b)

    SLOTS = NJ_CORE // 128
    in_maps = []
    for c in range(NCORES):
        j0 = c * NJ_CORE
        # wcoef[q, k*SLOTS + c] = W4[j0 + c*128 + q, k]
        wcoef = np.ascontiguousarray(
            W4[j0:j0 + NJ_CORE]
            .reshape(SLOTS, 128, 4)
            .transpose(1, 2, 0)
            .reshape(128, 4 * SLOTS)
        )
        in_maps.append(
            {
                "xt16": xt16,
                "wcoef": wcoef,
                "idxa16": _wrap_idx(idx_a[j0:j0 + NJ_CORE]),
                "idxb16": _wrap_idx(idx_b[j0:j0 + NJ_CORE]),
            }
        )
    return in_maps


_NC_CACHE = {}


def _get_nc():
    if "nc" not in _NC_CACHE:
        _NC_CACHE["nc"] = build_nc()
    return _NC_CACHE["nc"]


def _post(res, inputs=None):
    outt = np.concatenate([r["outt"] for r in res.results], axis=0)  # [OUT, B]
    return outt.T.astype(np.float32, order="C")


def kernel(x, weights, idx_a, idx_b):
    import sys

    if "/opt/trn_rl_repo" not in sys.path:
        sys.path.insert(0, "/opt/trn_rl_repo")
    from concourse.bass_utils import run_bass_kernel_spmd

    nc = _get_nc()
    in_maps = _prep_inputs(x, weights, idx_a, idx_b)
    res = run_bass_kernel_spmd(nc, in_maps, list(range(NCORES)))
    return _post(res)


if __name__ == "__main__":
    nc = build_nc()
    print("built OK")
